# revision 30
# baseline (speedup 1.0000x reference)
"""Trainium2 Bass kernel for DinoVisionTransformer Sparse-MoE FC2 (LoRA experts).

Computation (per token t):
    logits = x @ Wg                      -> top-2 softmax-renormalized weights
    out    = x @ W2 + b2 + sum_e cw[t,e] * scale[e] * (x @ A_e) @ B_e

Sharding: data-parallel over the batch dim (8 batch rows -> 8 NeuronCores,
1024 tokens each). All weights replicated.

Per-core kernel (mixed fp16 / fp8e4m3 compute, fp32 PSUM accumulation):
  The base FC2 (x @ W2) runs in fp16 (precision-critical: fp8 base alone
  gives ~3.6e-2 rel err, over the 2e-2 gate). The LoRA expert path (phase A
  x @ A_flat, phase B h @ Bm) and the router xlo correction run in fp8e4
  with DoubleRow perf mode (2 k-chunks per instruction, 2x PE rate).
  DoubleRow stationary loads (~136ns for 256 fp8 rows) do NOT hide behind
  another DoubleRow's short stream (113ns), so every DR matmul is
  interleaved between 512-col fp16 streams (213ns) that cover its load.

  Scaling scheme (all power-of-2, exact): W2, A, Bm are uploaded x64 so
  fp8/fp16 mantissas sit in the normal range. ps_base accumulates
  64*(x@W2); phase A produces ps_h = 64*h; the DVE combine weight is
  cw*scale/64 so hw = h*cw*scale; phase B adds hw@(64*Bm) = 64*delta into
  ps_base; final evac does out = ps_base/64 + b2 in one
  scalar_tensor_tensor.

  Router logits keep near-fp32 precision: logits = xtb@Wg_hi + xtb@Wg_lo
  (fp16) + 2^-17 * (xlo8 @ Wg8) where xlo8 = fp8((x - fp16(x)) * 2^11),
  Wg8 = fp8(Wg * 64). Top-2 of 8 via max8; w1 = sigmoid(l1-l2), w2 = 1-w1;
  dense combine weights by equality masks, then * scale/64.
"""

import sys

if "/opt/trn_rl_repo" not in sys.path:
    sys.path.insert(0, "/opt/trn_rl_repo")

import ml_dtypes
import numpy as np

import concourse.bass as bass  # noqa: F401  (registers types)
import concourse.mybir as mybir
import concourse.tile as tile
from concourse import bacc
from concourse.bass import ts
from concourse.bass_utils import run_bass_kernel_spmd
from concourse.masks import make_identity

P = 128
KCH = 32          # H / 128 contraction chunks
TT = 8            # 128-token tiles per core
H = 4096
D = 1024
E = 8
R = 64
ER = E * R        # 512
NW16 = D + 16     # wcat16 columns: W2*64 | Wg_hi | Wg_lo
NW8 = ER + 8      # acat8 columns: A_flat*64 | Wg8
NCORES = 8
WG_K_GROUPS = 8   # wcat16 DMA split granularity (k-chunks per group)
KPG = KCH // WG_K_GROUPS  # 4
ACH = 4           # acat8 DMA chunks (8 k-chunks each)

F16 = mybir.dt.float16
F32 = mybir.dt.float32
F8 = mybir.dt.float8e4
DR = mybir.MatmulPerfMode.DoubleRow

_CACHE = {}


def _build_nc():
    nc = bacc.Bacc("TRN2")

    xtb_d = nc.dram_tensor("xtb", [TT, P, KCH, P], F16, kind="ExternalInput")
    x8_d = nc.dram_tensor("x8", [TT, P, KCH, P], F8, kind="ExternalInput")
    xlo_d = nc.dram_tensor("xlo", [TT, P, KCH, P], F8, kind="ExternalInput")
    wcat_d = nc.dram_tensor("wcat", [P, KCH, NW16], F16, kind="ExternalInput")
    acat_d = nc.dram_tensor("acat", [P, KCH, NW8], F8, kind="ExternalInput")
    bm_d = nc.dram_tensor("bm", [P, 4, D], F8, kind="ExternalInput")
    b2b_d = nc.dram_tensor("b2b", [P, D], F32, kind="ExternalInput")
    sc_d = nc.dram_tensor("sc", [P, E], F32, kind="ExternalInput")
    y_d = nc.dram_tensor("y", [TT * P, D], F32, kind="ExternalOutput")

    Sig = mybir.ActivationFunctionType.Sigmoid
    Alu = mybir.AluOpType

    with tile.TileContext(nc) as tc:
        with (
            tc.tile_pool(name="wres", bufs=1) as wres,
            tc.tile_pool(name="xin", bufs=3) as xin,
            tc.tile_pool(name="small", bufs=2) as small,
            tc.tile_pool(name="hbuf", bufs=2) as hbuf,
            tc.tile_pool(name="obuf", bufs=2) as obuf,
            tc.tile_pool(name="ps_base", bufs=2, space="PSUM") as ps_base_pool,
            tc.tile_pool(name="ps_h", bufs=2, space="PSUM") as ps_h_pool,
            tc.tile_pool(name="ps_l", bufs=1, space="PSUM") as ps_l_pool,
            tc.tile_pool(name="ps_t", bufs=1, space="PSUM") as ps_t_pool,
        ):
            # ---- DMA issue order = consumption order (HWDGE drains roughly
            # in issue order). Phase A halves of tiles 0/1 fill the PE while
            # wcat16 streams in. ----
            xts = {}

            def alloc_x(t):
                xts[t] = (
                    xin.tile([P, KCH, P], F16, tag="xtb", name=f"xtb{t}"),
                    xin.tile([P, KCH, P], F8, tag="x8", name=f"x8_{t}"),
                    xin.tile([P, KCH, P], F8, tag="xlo", name=f"xlo{t}"),
                )

            def dma_x(t, which="all"):
                xtb_, x8_, xlo_ = xts[t]
                if which in ("all", "x8"):
                    nc.sync.dma_start(x8_[:], x8_d[t])
                if which in ("all", "xtb"):
                    nc.sync.dma_start(xtb_[:], xtb_d[t])
                if which in ("all", "xlo"):
                    nc.sync.dma_start(xlo_[:], xlo_d[t])

            alloc_x(0)
            alloc_x(1)
            acat_sb = []
            for c in range(ACH):
                acat_sb.append(
                    wres.tile([P, KCH // ACH, NW8], F8, tag=f"acat{c}",
                              name=f"acat{c}")
                )
            wcat_sb = []
            for g in range(WG_K_GROUPS):
                wcat_sb.append(
                    wres.tile([P, KPG, NW16], F16, tag=f"wcat{g}",
                              name=f"wcat{g}")
                )
            bm_sb = wres.tile([P, 4, D], F8, tag="bm")
            b2b_sb = wres.tile([P, D], F32, tag="b2b")
            sc_sb = wres.tile([P, E], F32, tag="sc")

            def dma_wg(g):
                nc.sync.dma_start(wcat_sb[g][:], wcat_d[:, ts(g, KPG), :])

            dma_x(0, "x8")
            nc.sync.dma_start(acat_sb[0][:], acat_d[:, ts(0, KCH // ACH), :])
            nc.sync.dma_start(acat_sb[1][:], acat_d[:, ts(1, KCH // ACH), :])
            dma_x(0, "xtb")
            dma_wg(0)
            nc.sync.dma_start(acat_sb[2][:], acat_d[:, ts(2, KCH // ACH), :])
            nc.sync.dma_start(acat_sb[3][:], acat_d[:, ts(3, KCH // ACH), :])
            dma_x(1, "x8")
            dma_wg(1)
            dma_x(1, "xtb")
            dma_wg(2)
            dma_wg(3)
            dma_x(0, "xlo")
            dma_wg(4)
            dma_x(1, "xlo")
            dma_wg(5)
            dma_wg(6)
            dma_wg(7)
            nc.sync.dma_start(bm_sb[:], bm_d[:])
            nc.sync.dma_start(b2b_sb[:], b2b_d[:])
            nc.sync.dma_start(sc_sb[:], sc_d[:])
            alloc_x(2)
            dma_x(2)
            alloc_x(3)
            dma_x(3)
            ident = wres.tile([P, P], F16, tag="ident")
            make_identity(nc, ident[:])

            def wc(k, lo, hi):
                return wcat_sb[k // KPG][:, k % KPG, lo:hi]

            def ac(kp, lo, hi):
                # k-pair kp covers k-chunks 2kp, 2kp+1; acat chunk c = kp//4
                c = kp // 4
                j = 2 * (kp % 4)
                return acat_sb[c][:, j:j + 2, lo:hi]

            # shared logits psum bank: tile t uses half (t % 2).
            # cols [0:16] = xtb @ [Wg_hi | Wg_lo]; cols [16:24] = 2^17x the
            # xlo correction (fp8 operands; rescaled on the DVE afterwards)
            ps_l_shared = ps_l_pool.tile([P, 64], F32, tag="l")

            pend = {}   # t -> (ps_base, ps_h, hwT or None)

            def lhalf(t):
                o = (t % 2) * 32
                return ps_l_shared[:, o:o + 32]

            def alloc_psums(t):
                pend[t] = (
                    ps_base_pool.tile([P, D], F32, tag="base", name=f"base{t}"),
                    ps_h_pool.tile([P, ER], F32, tag="h", name=f"h{t}"),
                    None,
                )
                # The shared logits bank must never see start=True (a bank-wide
                # has_written clear would wipe the other tile's half). Instead
                # zero this tile's half; start=False matmuls then accumulate
                # onto 0 (bits set) or overwrite with v (bits clear) — both ok.
                nc.vector.memset(lhalf(t)[:, 0:24], 0.0)

            def base3(t, k):
                """Base FC2 halves + router-hi for one k-chunk (fp16)."""
                xtb_sb, _, _ = xts[t]
                ps_base, _, _ = pend[t]
                ps_l = lhalf(t)
                st = k == 0
                nc.tensor.matmul(
                    ps_base[:, 0:512], xtb_sb[:, k, :], wc(k, 0, 512),
                    start=st, stop=False, skip_group_check=True,
                )
                nc.tensor.matmul(
                    ps_l[:, 0:16], xtb_sb[:, k, :], wc(k, D, NW16),
                    start=False, stop=(k == KCH - 1), skip_group_check=True,
                )
                nc.tensor.matmul(
                    ps_base[:, 512:1024], xtb_sb[:, k, :], wc(k, 512, 1024),
                    start=st, stop=False, skip_group_check=True,
                )

            def adr(t, kp):
                """One phase-A DoubleRow (k-chunks 2kp, 2kp+1)."""
                _, x8_sb, _ = xts[t]
                _, ps_h, _ = pend[t]
                nc.tensor.matmul(
                    ps_h[:, :], x8_sb[:, 2 * kp:2 * kp + 2, :],
                    ac(kp, 0, ER),
                    start=(kp == 0), stop=(kp == KCH // 2 - 1),
                    perf_mode=DR, skip_group_check=True,
                )

            def xlodr(t, kp):
                """One router-correction DoubleRow into ps_l[16:24]."""
                _, _, xlo_sb = xts[t]
                ps_l = lhalf(t)
                nc.tensor.matmul(
                    ps_l[:, 16:24], xlo_sb[:, 2 * kp:2 * kp + 2, :],
                    ac(kp, ER, ER + 8),
                    start=False, stop=(kp == KCH // 2 - 1),
                    perf_mode=DR, skip_group_check=True,
                )

            def b_dr(t, j):
                """Phase-B DoubleRow pair (er-chunks 2j,2j+1; both col halves,
                shared stationary so only one weight load)."""
                ps_base, _, hwT = pend[t]
                nc.tensor.matmul(
                    ps_base[:, 0:512], hwT[:, 2 * j:2 * j + 2, :],
                    bm_sb[:, 2 * j:2 * j + 2, 0:512],
                    start=False, stop=(j == 1),
                    perf_mode=DR, skip_group_check=True,
                )
                nc.tensor.matmul(
                    ps_base[:, 512:1024], hwT[:, 2 * j:2 * j + 2, :],
                    bm_sb[:, 2 * j:2 * j + 2, 512:1024],
                    start=False, stop=(j == 1),
                    perf_mode=DR, skip_group_check=True,
                )

            def evac(t):
                """Bias add (fp32, /64 rescale) + store of finished tile."""
                ps_base, _, _ = pend.pop(t)
                out_sb = obuf.tile([P, D], F32, tag="out")
                nc.vector.scalar_tensor_tensor(
                    out_sb[:], ps_base[:], 1.0 / 64.0, b2b_sb[:],
                    op0=Alu.mult, op1=Alu.add,
                )
                nc.scalar.dma_start(y_d[ts(t, P), :], out_sb[:])

            def emit_router_dve(t):
                """Router math + h-weighting (DVE/ACT only); returns hw."""
                _, ps_h, _ = pend[t]
                ps_l = lhalf(t)
                logits0 = small.tile([P, 8], F32, tag="logits0")
                nc.vector.tensor_reduce(
                    logits0[:],
                    ps_l[:, 0:16].rearrange("p (s j) -> p j s", s=2),
                    axis=mybir.AxisListType.X,
                    op=Alu.add,
                )
                logits = small.tile([P, 8], F32, tag="logits")
                nc.vector.scalar_tensor_tensor(
                    logits[:], ps_l[:, 16:24], 2.0 ** -17, logits0[:],
                    op0=Alu.mult, op1=Alu.add,
                )
                m8 = small.tile([P, 8], F32, tag="m8")
                nc.vector.max(m8[:], logits[:])
                g_ = small.tile([P, 1], F32, tag="gap")
                nc.vector.tensor_sub(g_[:], m8[:, 0:1], m8[:, 1:2])
                w1 = small.tile([P, 1], F32, tag="w1")
                nc.scalar.activation(w1[:], g_[:], Sig)
                w2 = small.tile([P, 1], F32, tag="w2")
                nc.scalar.activation(w2[:], g_[:], Sig, scale=-1.0)
                cw = small.tile([P, 8], F32, tag="cw")
                cwb = small.tile([P, 8], F32, tag="cwb")
                nc.vector.scalar_tensor_tensor(
                    cw[:], logits[:], m8[:, 0:1], w1[:, 0:1].to_broadcast([P, 8]),
                    op0=Alu.is_equal, op1=Alu.mult,
                )
                nc.vector.scalar_tensor_tensor(
                    cwb[:], logits[:], m8[:, 1:2], w2[:, 0:1].to_broadcast([P, 8]),
                    op0=Alu.is_equal, op1=Alu.mult,
                )
                nc.vector.tensor_add(cw[:], cw[:], cwb[:])
                # fold in scale[e]/64 (the /64 matches the x64 Bm upload)
                nc.vector.tensor_tensor(cw[:], cw[:], sc_sb[:], Alu.mult)
                hw = hbuf.tile([P, ER], F16, tag="hw")
                nc.vector.tensor_tensor(
                    hw.rearrange("p (e r) -> p e r", e=E),
                    ps_h.rearrange("p (e r) -> p e r", e=E),
                    cw[:, :, None].to_broadcast([P, E, R]),
                    Alu.mult,
                )
                return hw

            def emit_router_pe(t, hw):
                """PE transposes of weighted h + fp8 cast on copy-back."""
                ps_base, ps_h, _ = pend[t]
                ps_t = ps_t_pool.tile([P, ER], F16, tag="t")
                for j in range(4):
                    nc.tensor.transpose(
                        ps_t[:, ts(j, P)], hw[:, ts(j, P)], ident[:]
                    )
                hwT = hbuf.tile([P, 4, P], F8, tag="hwT")
                nc.vector.tensor_copy(hwT.rearrange("p a b -> p (a b)"), ps_t[:])
                pend[t] = (ps_base, ps_h, hwT)

            def a_half(t, h):
                """Bunched phase-A DRs, kp in [8h, 8h+8) (startup filler)."""
                for kp in range(8 * h, 8 * h + 8):
                    adr(t, kp)

            def group16(t, g):
                """One wcat group: base FC2 + router-hi per k-chunk.

                Same-type matmuls run bunched: the PE's weight-load pipeline
                only reaches full rate on homogeneous instruction runs —
                interleaving fp8 DoubleRows between fp16 streams measured
                SLOWER (every 512-col stream degraded to ~280-315ns)."""
                for k in range(g * KPG, (g + 1) * KPG):
                    base3(t, k)

            def emit_xlo(t):
                """Router xlo-correction block (16 narrow DoubleRows)."""
                for kp in range(KCH // 2):
                    xlodr(t, kp)

            # ---- startup: phase A of tiles 0/1 (bunched) fills the PE while
            # wcat16 streams in fine 2-k-chunk groups; base groups then
            # interleave tiles 0/1, tile 0 running two groups ahead ----
            alloc_psums(0)
            alloc_psums(1)
            a_half(0, 0)
            group16(0, 0)
            a_half(0, 1)
            a_half(1, 0)
            a_half(1, 1)
            for g in range(1, WG_K_GROUPS):
                group16(0, g)
                group16(1, g - 1)
            emit_xlo(0)
            hw0 = emit_router_dve(0)
            group16(1, WG_K_GROUPS - 1)
            emit_router_pe(0, hw0)
            emit_xlo(1)
            b_dr(0, 0)
            b_dr(0, 1)
            evac(0)
            hw_pend = {1: emit_router_dve(1)}

            # ---- steady state: bunched A block, base groups, xlo block ----
            for t in range(2, TT - 1):
                if t >= 4:
                    alloc_x(t)
                    dma_x(t)
                alloc_psums(t)
                a_half(t, 0)
                a_half(t, 1)
                for g in range(WG_K_GROUPS):
                    group16(t, g)
                    # transposes after g0 (not before): gives the previous
                    # tile's DVE router chain ~1.8us more headroom before the
                    # PE needs its hw result (matters at the t=2 boundary
                    # where that chain is queued behind startup DVE work)
                    if g == 0 and (t - 1) in hw_pend:
                        emit_router_pe(t - 1, hw_pend.pop(t - 1))
                    if g == 4:
                        b_dr(t - 1, 0)
                        b_dr(t - 1, 1)
                    if g == 5:
                        evac(t - 1)
                emit_xlo(t)
                hw_pend[t] = emit_router_dve(t)

            # ---- last tile: bunched A/L16/xlo prefix so the router DVE
            # chain overlaps the base k-loop; transposes injected mid-loop;
            # phase B tail with split evac ----
            t = TT - 1
            alloc_x(t)
            dma_x(t)
            alloc_psums(t)
            xtb_sb, _, _ = xts[t]
            ps_base, ps_h, _ = pend[t]
            ps_l = lhalf(t)
            a_half(t, 0)
            a_half(t, 1)
            emit_router_pe(t - 1, hw_pend.pop(t - 1))
            for k in range(KCH):
                nc.tensor.matmul(
                    ps_l[:, 0:16], xtb_sb[:, k, :], wc(k, D, NW16),
                    start=False, stop=(k == KCH - 1),
                    skip_group_check=True,
                )
            emit_xlo(t)
            hw_last = emit_router_dve(t)
            for k in range(KCH):
                st = k == 0
                fin = k == KCH - 1
                nc.tensor.matmul(
                    ps_base[:, 0:512], xtb_sb[:, k, :], wc(k, 0, 512),
                    start=st, stop=fin, skip_group_check=True,
                )
                nc.tensor.matmul(
                    ps_base[:, 512:1024], xtb_sb[:, k, :], wc(k, 512, 1024),
                    start=st, stop=fin, skip_group_check=True,
                )
                if k == 4:
                    b_dr(t - 1, 0)
                if k == 6:
                    b_dr(t - 1, 1)
                if k == 8:
                    evac(t - 1)
                if k == 10:
                    emit_router_pe(t, hw_last)
                # tile 7's own phase B injected late in the base loop (hwT
                # ready by k~14) so the post-loop drain is just the evacs;
                # base k==31 carries the accumulation-group stop flags
                if k == 24 or k == 28:
                    lo = 0 if k == 24 else 512
                    _, _, hwT7 = pend[t]
                    for j in range(2):
                        nc.tensor.matmul(
                            ps_base[:, lo:lo + 512],
                            hwT7[:, 2 * j:2 * j + 2, :],
                            bm_sb[:, 2 * j:2 * j + 2, lo:lo + 512],
                            start=False, stop=False,
                            perf_mode=DR, skip_group_check=True,
                        )
            # drain: split evac; first half's bias-add + store overlap the
            # second half's final base matmul retire
            ps_base, _, _ = pend.pop(t)
            out_sb = obuf.tile([P, D], F32, tag="out")
            nc.vector.scalar_tensor_tensor(
                out_sb[:, 0:512], ps_base[:, 0:512], 1.0 / 64.0,
                b2b_sb[:, 0:512], op0=Alu.mult, op1=Alu.add,
            )
            nc.scalar.dma_start(y_d[ts(t, P), 0:512], out_sb[:, 0:512])
            nc.vector.scalar_tensor_tensor(
                out_sb[:, 512:1024], ps_base[:, 512:1024], 1.0 / 64.0,
                b2b_sb[:, 512:1024], op0=Alu.mult, op1=Alu.add,
            )
            nc.scalar.dma_start(y_d[ts(t, P), 512:1024], out_sb[:, 512:1024])

    nc.finalize()
    return nc


F8NP = ml_dtypes.float8_e4m3


def _prep_shared(Wg, W2, b2, A, Bm, scale):
    """Host-side weight layout prep (replicated across cores)."""
    f16, f32 = np.float16, np.float32
    # wcat16 = [W2*64 | Wg_hi | Wg_lo], k-chunked to [128, 32, NW16]
    wg_hi = Wg.astype(f16)
    wg_lo = (Wg.astype(f32) - wg_hi.astype(f32)).astype(f16)
    wcat = np.empty((H, NW16), dtype=f16)
    wcat[:, 0:D] = (W2.astype(f32) * 64.0).astype(f16)
    wcat[:, D:D + 8] = wg_hi
    wcat[:, D + 8:] = wg_lo
    wcat = np.ascontiguousarray(wcat.reshape(KCH, P, NW16).transpose(1, 0, 2))

    # acat8 = [A_flat*64 | Wg*64], k-chunked to [128, 32, NW8] fp8
    a_flat = np.ascontiguousarray(A.transpose(1, 0, 2)).reshape(H, ER)
    acat = np.empty((H, NW8), dtype=F8NP)
    acat[:, 0:ER] = (a_flat.astype(f32) * 64.0).astype(F8NP)
    acat[:, ER:] = (Wg.astype(f32) * 64.0).astype(F8NP)
    acat = np.ascontiguousarray(acat.reshape(KCH, P, NW8).transpose(1, 0, 2))

    # Bm*64 (scale NOT folded; it rides in the DVE combine weights),
    # [(e r), d] -> [128, 4, D] fp8
    bms = (Bm.astype(f32) * 64.0).reshape(ER, D)
    bms = np.ascontiguousarray(
        bms.reshape(4, P, D).transpose(1, 0, 2)
    ).astype(F8NP)

    b2b = np.ascontiguousarray(
        np.broadcast_to(b2.astype(f32)[None, :], (P, D))
    )
    scb = np.ascontiguousarray(
        np.broadcast_to((scale.astype(f32) / 64.0)[None, :], (P, E))
    )
    return wcat, acat, bms, b2b, scb


def _prep_x_core(x_c):
    """Per-core x prep: fp16 + fp8 copies + scaled-fp8 lo residual,
    [tile, p, k, ti] layout."""
    f16, f32 = np.float16, np.float32
    xtb = x_c.astype(f16)                                   # [1024, 4096]
    x8 = x_c.astype(F8NP)
    xlo = ((x_c.astype(f32) - xtb.astype(f32)) * 2048.0).astype(F8NP)

    def lay(a):
        return np.ascontiguousarray(
            a.reshape(TT, P, KCH, P).transpose(0, 3, 2, 1)
        )
    return lay(xtb), lay(x8), lay(xlo)


def kernel(x, Wg, W2, b2, A, Bm, scale):
    x = np.asarray(x, dtype=np.float32)
    Wg = np.asarray(Wg, dtype=np.float32)
    W2 = np.asarray(W2, dtype=np.float32)
    b2 = np.asarray(b2, dtype=np.float32)
    A = np.asarray(A, dtype=np.float32)
    Bm = np.asarray(Bm, dtype=np.float32)
    scale = np.asarray(scale, dtype=np.float32)

    if "nc" not in _CACHE:
        _CACHE["nc"] = _build_nc()
    nc = _CACHE["nc"]

    wcat, acat, bms, b2b, scb = _prep_shared(Wg, W2, b2, A, Bm, scale)
    in_maps = []
    for c in range(NCORES):
        xtb, x8, xlo = _prep_x_core(x[c])
        in_maps.append({
            "xtb": xtb, "x8": x8, "xlo": xlo, "wcat": wcat,
            "acat": acat, "bm": bms, "b2b": b2b, "sc": scb,
        })

    res = run_bass_kernel_spmd(nc, in_maps, core_ids=list(range(NCORES)))
    out = np.stack([res.results[c]["y"] for c in range(NCORES)], axis=0)
    return out.astype(np.float32)
